# revision 1
# baseline (speedup 1.0000x reference)
"""GCN-Tox21 GNN message-passing kernel for 8 Trainium2 NeuronCores.

Strategy (graph/edge parallelism):
  - Sort edges by destination node on the host; core k owns the destination
    node range [k*NPC, (k+1)*NPC) and all edges pointing into it.
  - Node features h live replicated in each core's DRAM (bf16). Per-edge
    gathers of h[dst], h[src] use dma_gather(transpose=True), which lands
    features feature-major in SBUF, ready as matmul moving operands.
  - The per-edge 2-layer MLP runs on the tensor engine in bf16 with fp32
    PSUM accumulation. b1 is folded in via a constant-ones row appended to
    the e^T operand; b2 is added with a broadcast tile on the vector engine.
  - Segment-sum to destination nodes is a matmul with host-built 0/1 one-hot
    tiles (edges sorted by dst => each 128-node window's edges are
    contiguous; PSUM accumulates across the window's edge subtiles).
  - Mean + eval-mode BN fold into h = relu((seg_sum + cnt'*cb) * invcnt),
    cb = bn_b - bn_m*A, A = g/sqrt(rv+eps); w2/b2 pre-scaled by A. The
    rank-1 cnt'*cb term is one K=1 matmul per window.
  - After each conv layer an AllGather rebuilds the replicated h.
  - Mean-pool + FC + sigmoid: core k handles graphs [k*GPC, (k+1)*GPC)
    (batch is sorted, so their nodes are contiguous; dma_gather fetches
    them node-major for the pooling matmul).
"""

import numpy as np
import ml_dtypes

import concourse.bacc as bacc
import concourse.tile as tile
from concourse import mybir, bass_utils
from concourse.masks import make_identity

BF16 = mybir.dt.bfloat16
F32 = mybir.dt.float32
I16 = mybir.dt.int16
RELU = mybir.ActivationFunctionType.Relu

N_CORES = 8
BN_EPS = 1e-5
G_REAL = 512
F_NODE, F_EDGE, H, EH = 32, 8, 256, 16
OUT_DIMS = (256, 256, 128)
EG = 512  # edges per gather batch


def _bf(a):
    return np.ascontiguousarray(a.astype(ml_dtypes.bfloat16))


def _f32(a):
    return np.ascontiguousarray(a.astype(np.float32))


def _wrap_idx(idx):
    """int16 index layout for dma_gather: index i at [i % 16, i // 16],
    replicated across the 8 partition groups."""
    assert len(idx) % 16 == 0
    w = idx.astype(np.int16).reshape(-1, 16).T
    return np.ascontiguousarray(np.tile(w, (8, 1)))


class Plan:
    """Host-side preprocessing: sharding layout + per-core input tensors."""

    def __init__(self, inputs, G):
        x = np.asarray(inputs["x"]).astype(np.float32)
        N = x.shape[0]
        self.N, self.G = N, G
        self.N_pad = ((N + N_CORES * 128 - 1) // (N_CORES * 128)) * (N_CORES * 128)
        self.NPC = self.N_pad // N_CORES
        self.W = self.NPC // 128
        assert G % N_CORES == 0
        self.GPC = G // N_CORES

        edge_index = np.asarray(inputs["edge_index"]).astype(np.int64)
        src, dst = edge_index[0].astype(np.int32), edge_index[1].astype(np.int32)
        batch = np.asarray(inputs["batch"]).astype(np.int32)
        edge_attr = np.asarray(inputs["edge_attr"]).astype(np.float32)

        order = np.argsort(dst, kind="stable")
        s_dst, s_src = dst[order], src[order]
        s_ea = edge_attr[order]

        bounds = np.searchsorted(s_dst, np.arange(0, self.N_pad + 1, 128), "left")
        cnt_w = (bounds[1:] - bounds[:-1]).reshape(N_CORES, self.W)
        T_w = np.maximum(1, -(-cnt_w.max(axis=0) // 128))
        while T_w.sum() % (EG // 128) != 0:
            T_w[-1] += 1
        self.T_w = [int(t) for t in T_w]
        self.T_tot = int(T_w.sum())
        self.ET = self.T_tot * 128

        cnt = np.bincount(dst, minlength=self.N_pad).astype(np.float32)
        invc_full = 1.0 / np.maximum(cnt, 1.0)
        cntp_full = np.maximum(cnt, 1.0)
        gcnt = np.bincount(batch, minlength=G).astype(np.float32)
        ginv_full = 1.0 / np.maximum(gcnt, 1.0)

        lo_k = [int(np.searchsorted(batch, k * self.GPC, "left")) for k in range(N_CORES)]
        hi_k = [int(np.searchsorted(batch, (k + 1) * self.GPC, "left")) for k in range(N_CORES)]
        self.TP = max(1, max(-(-(h - l) // 128) for l, h in zip(lo_k, hi_k)))
        self.NPOOL = self.TP * 128

        self.per_core = []
        for k in range(N_CORES):
            d = {}
            gi_dst = np.zeros(self.ET, np.int32)
            gi_src = np.zeros(self.ET, np.int32)
            ea_pad = np.zeros((self.ET, F_EDGE), np.float32)
            S = np.zeros((128, self.ET), np.float32)
            pos = 0
            for w in range(self.W):
                base = k * self.NPC + w * 128
                lo = np.searchsorted(s_dst, base, "left")
                hi = np.searchsorted(s_dst, base + 128, "left")
                n = hi - lo
                sl = slice(pos, pos + n)
                gi_dst[sl] = s_dst[lo:hi]
                gi_src[sl] = s_src[lo:hi]
                ea_pad[sl] = s_ea[lo:hi]
                loc = (s_dst[lo:hi] - base).astype(np.int64)
                e_ids = np.arange(pos, pos + n)
                S[e_ids % 128, (e_ids // 128) * 128 + loc] = 1.0
                pos += self.T_w[w] * 128
            assert pos == self.ET

            d["gidx_src"] = _wrap_idx(gi_src)
            d["S"] = _bf(S)
            # transposed one-hot: S_T[n, t*128+p] = S[p, t*128+n]
            ST = np.ascontiguousarray(
                S.reshape(128, self.T_tot, 128).transpose(2, 1, 0)
                .reshape(128, self.ET))
            d["S_T"] = _bf(ST)
            eaT = np.concatenate([ea_pad.T, np.ones((1, self.ET), np.float32)], 0)
            d["eaT"] = _bf(eaT)
            stripe = slice(k * self.NPC, (k + 1) * self.NPC)
            d["invc"] = _f32(invc_full[stripe].reshape(self.W, 128).T)
            d["cntrow"] = _bf(cntp_full[stripe].reshape(1, self.NPC))
            lo, hi = lo_k[k], hi_k[k]
            pidx = np.zeros(self.NPOOL, np.int32)
            pidx[: hi - lo] = np.arange(lo, hi)
            d["pool_idx"] = _wrap_idx(pidx)
            S2 = np.zeros((128, self.TP * self.GPC), np.float32)
            pb = batch[lo:hi] - k * self.GPC
            e_ids = np.arange(hi - lo)
            S2[e_ids % 128, (e_ids // 128) * self.GPC + pb] = 1.0
            d["S2"] = _bf(S2)
            d["ginv"] = _f32(ginv_full[k * self.GPC:(k + 1) * self.GPC].reshape(self.GPC, 1))
            self.per_core.append(d)

        sh = {}
        x_pad = np.zeros((self.N_pad, F_NODE), np.float32)
        x_pad[:N] = x
        xT_full = np.concatenate([x_pad.T, np.ones((1, self.N_pad), np.float32)], 0)
        sh["xT"] = _bf(xT_full)
        for k in range(N_CORES):
            self.per_core[k]["xT_own"] = _bf(
                xT_full[:, k * self.NPC:(k + 1) * self.NPC])
        ne_w, ne_b = _f32(inputs["ne_w"]), _f32(inputs["ne_b"])
        sh["ne_wT"] = _bf(np.concatenate([ne_w.T, ne_b[None, :]], 0))
        ee_w, ee_b = _f32(inputs["ee_w"]), _f32(inputs["ee_b"])
        sh["ee_wT"] = _bf(np.concatenate([ee_w.T, ee_b[None, :]], 0))

        in_dim = H
        self.layer_dims = []
        for i, out_dim in enumerate(OUT_DIMS):
            w1 = _f32(inputs[f"c{i}_w1"]); b1 = _f32(inputs[f"c{i}_b1"])
            w2 = _f32(inputs[f"c{i}_w2"]); b2 = _f32(inputs[f"c{i}_b2"])
            g = _f32(inputs[f"bn{i}_g"]); bb = _f32(inputs[f"bn{i}_b"])
            rm = _f32(inputs[f"bn{i}_m"]); rv = _f32(inputs[f"bn{i}_v"])
            A = g / np.sqrt(rv + BN_EPS)
            F_mid = 2 * out_dim
            # K-order: [h_dst(in), h_src(in), e(EH), ones]
            sh[f"w1T_{i}"] = _bf(np.concatenate([w1.T, b1[None, :]], 0))
            sh[f"w2T_{i}"] = _bf((w2 * A[:, None]).T)
            sh[f"b2bc_{i}"] = _f32(np.tile((b2 * A)[None, :], (128, 1)))
            sh[f"cbrow_{i}"] = _bf((bb - rm * A)[None, :])
            self.layer_dims.append((in_dim, F_mid, out_dim))
            in_dim = out_dim

        fc_w, fc_b = _f32(inputs["fc_w"]), _f32(inputs["fc_b"])
        self.F_FC = fc_w.shape[0]
        sh["fc_wT"] = _bf(fc_w.T)
        sh["fcb_bc"] = _f32(np.tile(fc_b[None, :], (self.GPC, 1)))
        self.shared = sh

    def in_maps(self):
        return [{**self.shared, **self.per_core[k]} for k in range(N_CORES)]


def build_program(plan: Plan, n_cores=N_CORES, debug_no_collective=False,
                  debug_stage=9, repeats=1, skip_gather=False, skip_compute=False):
    nc = bacc.Bacc("TRN2", target_bir_lowering=False, debug=False,
                   num_devices=n_cores)

    ET, T_w, W, NPC, TP, GPC = plan.ET, plan.T_w, plan.W, plan.NPC, plan.TP, plan.GPC
    N_pad, NPOOL, F_FC = plan.N_pad, plan.NPOOL, plan.F_FC

    sample = plan.in_maps()[0]
    t_in = {name: nc.dram_tensor(name, list(arr.shape),
                                 mybir.dt.from_np(arr.dtype), kind="ExternalInput")
            for name, arr in sample.items()}
    out_part = nc.dram_tensor("out_part", [GPC, F_FC], F32, kind="ExternalOutput")

    n_batches = ET // EG if ET % EG == 0 else ET // EG + 1

    # subtile -> window mapping (static)
    sub_window, sub_first, sub_last = [], [], []
    for w in range(W):
        for t in range(T_w[w]):
            sub_window.append(w)
            sub_first.append(t == 0)
            sub_last.append(t == T_w[w] - 1)

    with tile.TileContext(nc) as tc:
        with (
            tc.tile_pool(name="const", bufs=1) as cpool,
            tc.tile_pool(name="sbuf", bufs=2) as spool,
            tc.tile_pool(name="gath", bufs=8) as gpool,
            tc.tile_pool(name="m1sb", bufs=8) as m1pool,
            tc.tile_pool(name="psum", bufs=2, space="PSUM") as ppool,
            tc.tile_pool(name="dram", bufs=1, space="DRAM") as dpool,
        ):
            def _body():
                # ---------- resident constants ----------
                def load_const(name, dtype=None, tag=None):
                    arr = sample[name]
                    t = cpool.tile(list(arr.shape), dtype or mybir.dt.from_np(arr.dtype),
                                   tag=tag or name)
                    nc.sync.dma_start(out=t[:], in_=t_in[name][:])
                    return t

                ST_t = load_const("S_T")
                gidx_src = load_const("gidx_src")
                pool_idx = load_const("pool_idx")
                invc_t = load_const("invc")
                cntrow_t = load_const("cntrow")
                S2_t = load_const("S2")
                ginv_t = load_const("ginv")
                ne_wT_t = load_const("ne_wT")
                ee_wT_t = load_const("ee_wT")
                fc_wT_t = load_const("fc_wT")
                fcb_t = load_const("fcb_bc")
                ident = cpool.tile([128, 128], BF16, tag="ident")
                make_identity(nc, ident[:])

                w1T_t, w2T_t, b2bc_t, cbrow_t = [], [], [], []
                for i, (F_in, F_mid, F_out) in enumerate(plan.layer_dims):
                    KC2 = 2 * F_in // 128
                    chunks = []
                    for kc in range(KC2):
                        t = cpool.tile([128, F_mid], BF16, tag=f"w1T_{i}_{kc}")
                        nc.sync.dma_start(out=t[:],
                                          in_=t_in[f"w1T_{i}"][kc * 128:(kc + 1) * 128, :])
                        chunks.append(t)
                    te = cpool.tile([EH + 1, F_mid], BF16, tag=f"w1Te_{i}")
                    nc.sync.dma_start(out=te[:],
                                      in_=t_in[f"w1T_{i}"][2 * F_in:2 * F_in + EH + 1, :])
                    w1T_t.append((chunks, te))
                    wc = []
                    for km in range(F_mid // 128):
                        t = cpool.tile([128, F_out], BF16, tag=f"w2T_{i}_{km}")
                        nc.sync.dma_start(out=t[:],
                                          in_=t_in[f"w2T_{i}"][km * 128:(km + 1) * 128, :])
                        wc.append(t)
                    w2T_t.append(wc)
                    b2bc_t.append(load_const(f"b2bc_{i}"))
                    cbrow_t.append(load_const(f"cbrow_{i}"))
                hT_t = [cpool.tile([128, W, F_in_ // 128, 128], BF16, tag=f"hT{i}",
                                   name=f"hT{i}")
                        for i, (F_in_, _, _) in enumerate(plan.layer_dims)]

                # ---------- DRAM buffers ----------
                h_full = [dpool.tile([N_pad, plan.layer_dims[0][0]], BF16, tag="h0",
                                     name="h_full0")]
                for i, (_, _, F_out) in enumerate(plan.layer_dims):
                    h_full.append(dpool.tile([N_pad, F_out], BF16, tag=f"h{i + 1}",
                                             name=f"h_full{i + 1}"))
                h_own = [dpool.tile([NPC, d[2]], BF16, tag=f"hown{i}",
                                    name=f"h_own{i}")
                         for i, d in enumerate(plan.layer_dims)]
                eT_dram = dpool.tile([EH + 1, ET], BF16, tag="eT")

                # ---------- stage A: h0 = relu(x @ ne_w.T + ne_b) (replicated) ----------
                for chunk in range(N_pad // 128):
                    n0 = chunk * 128
                    xt = spool.tile([F_NODE + 1, 128], BF16, tag="xT")
                    nc.sync.dma_start(out=xt[:], in_=t_in["xT"][:, n0:n0 + 128])
                    ps = ppool.tile([128, H], F32, tag="m2")
                    nc.tensor.matmul(out=ps[:], lhsT=xt[:], rhs=ne_wT_t[:],
                                     start=True, stop=True)
                    h0sb = spool.tile([128, H], BF16, tag="h0sb")
                    nc.scalar.activation(out=h0sb[:], in_=ps[:], func=RELU)
                    nc.sync.dma_start(out=h_full[0][n0:n0 + 128, :], in_=h0sb[:])
                # own-stripe h0 again, transposed into hT_t[0] for the dst path
                for w in range(W):
                    xo = spool.tile([F_NODE + 1, 128], BF16, tag="xT")
                    nc.sync.dma_start(out=xo[:],
                                      in_=t_in["xT_own"][:, w * 128:(w + 1) * 128])
                    ps = ppool.tile([128, H], F32, tag="m2")
                    nc.tensor.matmul(out=ps[:], lhsT=xo[:], rhs=ne_wT_t[:],
                                     start=True, stop=True)
                    h0o = spool.tile([128, H], BF16, tag="h0sb")
                    nc.scalar.activation(out=h0o[:], in_=ps[:], func=RELU)
                    for kc in range(H // 128):
                        tp = ppool.tile([128, 128], BF16, tag="m2")
                        nc.tensor.transpose(out=tp[:],
                                            in_=h0o[:, kc * 128:(kc + 1) * 128],
                                            identity=ident[:])
                        nc.vector.tensor_copy(out=hT_t[0][:, w, kc, :], in_=tp[:])

                # ---------- stage A2: e^T (+ones row) -> DRAM [EH+1, ET] ----------
                if debug_stage < 2:
                    return
                ones_row = cpool.tile([1, 512], BF16, tag="ones512")
                nc.vector.memset(ones_row[:], 1.0)
                for g0 in range(ET // 512):
                    ea_t = spool.tile([F_EDGE + 1, 512], BF16, tag="eaT")
                    nc.sync.dma_start(out=ea_t[:],
                                      in_=t_in["eaT"][:, g0 * 512:(g0 + 1) * 512])
                    ps = ppool.tile([EH, 512], F32, tag="m2")
                    nc.tensor.matmul(out=ps[:], lhsT=ee_wT_t[:], rhs=ea_t[:],
                                     start=True, stop=True)
                    et_sb = spool.tile([EH, 512], BF16, tag="etsb")
                    nc.scalar.activation(out=et_sb[:], in_=ps[:], func=RELU)
                    nc.sync.dma_start(out=eT_dram[0:EH, g0 * 512:(g0 + 1) * 512],
                                      in_=et_sb[:])
                    nc.sync.dma_start(out=eT_dram[EH:EH + 1, g0 * 512:(g0 + 1) * 512],
                                      in_=ones_row[:])

                # ---------- conv layers ----------
                if debug_stage < 3:
                    return
                for li, (F_in, F_mid, F_out) in enumerate(plan.layer_dims):
                    h_in = h_full[li]
                    KC = F_in // 128
                    MC = F_mid // 128
                    node_ps = None
                    qt_by_window = {}
                    for b in range(n_batches):
                        e0 = b * EG
                        eg = min(EG, ET - e0)
                        gs = gpool.tile([128, KC, eg], BF16, tag="gs")
                        if not skip_gather:
                            nc.gpsimd.dma_gather(gs[:], h_in[:, :],
                                                 gidx_src[:, e0 // 16:(e0 + eg) // 16],
                                                 eg, eg, F_in, transpose=True)
                        for gsub in (range(0) if skip_compute else range(eg // 512)):
                            g = (e0 + gsub * 512) // 512
                            c0 = gsub * 512
                            et_t = spool.tile([EH + 1, 512], BF16, tag="et_in", bufs=4)
                            nc.sync.dma_start(out=et_t[:],
                                              in_=eT_dram[:, g * 512:(g + 1) * 512])
                            # per-window Q^T = h_win @ W1d.T, expanded per edge below
                            for s in range(4):
                                t_glob = g * 4 + s
                                if sub_first[t_glob]:
                                    w = sub_window[t_glob]
                                    qtp = ppool.tile([128, F_mid], F32, tag="qt")
                                    for kc in range(KC):
                                        nc.tensor.matmul(
                                            out=qtp[:], lhsT=hT_t[li][:, w, kc, :],
                                            rhs=w1T_t[li][0][kc][:],
                                            start=(kc == 0), stop=(kc == KC - 1))
                                    qsb = spool.tile([128, F_mid], BF16, tag="qt_sb",
                                                     bufs=3)
                                    nc.vector.tensor_copy(out=qsb[:], in_=qtp[:])
                                    qt_by_window[w] = qsb
                            m1sb = []
                            for fo in range(MC):
                                ps = ppool.tile([128, 512], F32, tag="m1")
                                fsl = slice(fo * 128, (fo + 1) * 128)
                                for kc in range(KC):
                                    nc.tensor.matmul(
                                        out=ps[:], lhsT=w1T_t[li][0][KC + kc][:, fsl],
                                        rhs=gs[:, kc, c0:c0 + 512],
                                        start=(kc == 0), stop=False,
                                        skip_group_check=True)
                                s = 0
                                while s < 4:
                                    t_glob = g * 4 + s
                                    w0 = sub_window[t_glob]
                                    s2 = s
                                    while s2 + 1 < 4 and sub_window[g * 4 + s2 + 1] == w0:
                                        s2 += 1
                                    qsb = qt_by_window[w0]
                                    nc.tensor.matmul(
                                        out=ps[:, s * 128:(s2 + 1) * 128],
                                        lhsT=qsb[:, fsl],
                                        rhs=ST_t[:, t_glob * 128:
                                                 (g * 4 + s2 + 1) * 128],
                                        start=False, stop=False,
                                        skip_group_check=True)
                                    s = s2 + 1
                                nc.tensor.matmul(
                                    out=ps[:], lhsT=w1T_t[li][1][:, fsl], rhs=et_t[:],
                                    start=False, stop=True, skip_group_check=True)
                                sb = m1pool.tile([128, 512], BF16, tag="m1sb")
                                nc.scalar.activation(out=sb[:], in_=ps[:], func=RELU)
                                m1sb.append(sb)
                            for s in range(4):
                                t_glob = g * 4 + s
                                w = sub_window[t_glob]
                                ps2 = ppool.tile([128, F_out], F32, tag="m2")
                                esl = slice(s * 128, (s + 1) * 128)
                                for km in range(MC):
                                    nc.tensor.matmul(
                                        out=ps2[:], lhsT=m1sb[km][:, esl],
                                        rhs=w2T_t[li][km][:],
                                        start=(km == 0), stop=(km == MC - 1))
                                m2sb = spool.tile([128, F_out], BF16, tag="m2sb")
                                nc.vector.tensor_tensor(out=m2sb[:], in0=ps2[:],
                                                        in1=b2bc_t[li][:],
                                                        op=mybir.AluOpType.add)
                                st_t = spool.tile([128, 128], BF16, tag="s_in",
                                                  bufs=6)
                                nc.sync.dma_start(
                                    out=st_t[:],
                                    in_=t_in["S"][:, t_glob * 128:(t_glob + 1) * 128])
                                if sub_first[t_glob]:
                                    node_ps = ppool.tile([128, F_out], F32, tag="node")
                                    nc.tensor.matmul(
                                        out=node_ps[:],
                                        lhsT=cntrow_t[0:1, w * 128:(w + 1) * 128],
                                        rhs=cbrow_t[li][:], start=True, stop=False,
                                        skip_group_check=True)
                                nc.tensor.matmul(
                                    out=node_ps[:],
                                    lhsT=st_t[:],
                                    rhs=m2sb[:], start=False, stop=sub_last[t_glob],
                                    skip_group_check=True)
                                if sub_last[t_glob]:
                                    hsb = spool.tile([128, F_out], BF16, tag="hsb")
                                    nc.scalar.activation(out=hsb[:], in_=node_ps[:],
                                                         func=RELU,
                                                         scale=invc_t[:, w:w + 1])
                                    nc.sync.dma_start(
                                        out=h_own[li][w * 128:(w + 1) * 128, :],
                                        in_=hsb[:])
                                    if li < 2:
                                        for kc in range(F_out // 128):
                                            tp = ppool.tile([128, 128], BF16,
                                                            tag="m2")
                                            nc.tensor.transpose(
                                                out=tp[:],
                                                in_=hsb[:, kc * 128:(kc + 1) * 128],
                                                identity=ident[:])
                                            nc.vector.tensor_copy(
                                                out=hT_t[li + 1][:, w, kc, :],
                                                in_=tp[:])
                    if debug_no_collective:
                        cp = spool.tile([128, F_out], BF16, tag="dbgcp")
                        nc.sync.dma_start(out=cp[:], in_=h_own[li][0:128, :])
                        nc.sync.dma_start(out=h_full[li + 1][0:128, :], in_=cp[:])
                    else:
                        nc.gpsimd.collective_compute(
                            "AllGather", mybir.AluOpType.bypass,
                            ins=[h_own[li].opt()], outs=[h_full[li + 1].opt()],
                            replica_groups=[list(range(n_cores))])

                # ---------- pooling + FC + sigmoid ----------
                if debug_stage < 5:
                    return
                F_last = plan.layer_dims[-1][2]
                hp = spool.tile([128, TP, F_last], BF16, tag="hp")
                # gather in <=512-index chunks (larger single gathers crash)
                for p0 in range(0, TP, 4):
                    pn = min(4, TP - p0)
                    nc.gpsimd.dma_gather(
                        hp[:, p0:p0 + pn, :], h_full[-1][:, :],
                        pool_idx[:, p0 * 8:(p0 + pn) * 8],
                        pn * 128, pn * 128, F_last, transpose=False)
                if debug_stage < 6:
                    return
                pool_ps = ppool.tile([GPC, F_last], F32, tag="m1")
                for t in range(TP):
                    nc.tensor.matmul(out=pool_ps[:],
                                     lhsT=S2_t[:, t * GPC:(t + 1) * GPC],
                                     rhs=hp[:, t, :], start=(t == 0), stop=(t == TP - 1))
                pooled_sb = spool.tile([GPC, F_last], BF16, tag="pooled")
                nc.scalar.activation(out=pooled_sb[:], in_=pool_ps[:],
                                     func=mybir.ActivationFunctionType.Copy,
                                     scale=ginv_t[:])
                if debug_stage < 7:
                    return
                ptr_ps = ppool.tile([F_last, GPC], BF16, tag="qt")
                nc.tensor.transpose(out=ptr_ps[:], in_=pooled_sb[:],
                                    identity=ident[0:GPC, 0:GPC])
                ptr_sb = spool.tile([F_last, GPC], BF16, tag="ptrsb")
                nc.vector.tensor_copy(out=ptr_sb[:], in_=ptr_ps[:])
                if debug_stage < 8:
                    return
                fc_ps = ppool.tile([GPC, F_FC], F32, tag="node")
                nc.tensor.matmul(out=fc_ps[:], lhsT=ptr_sb[:], rhs=fc_wT_t[:],
                                 start=True, stop=True)
                logit = spool.tile([GPC, F_FC], F32, tag="logit")
                nc.vector.tensor_tensor(out=logit[:], in0=fc_ps[:], in1=fcb_t[:],
                                        op=mybir.AluOpType.add)
                # Sigmoid's activation-table load (sigmoid_and_friends) crashes
                # this runtime; the host applies the exact fp32 sigmoid instead.
                nc.sync.dma_start(out=out_part[:], in_=logit[:])

            for _r in range(repeats):
                _body()

    nc.compile()
    return nc


_CACHE = {}


def run(inputs, G=G_REAL):
    plan = Plan(inputs, G)
    key = (plan.N, plan.G, plan.TP, tuple(plan.T_w))
    if key not in _CACHE:
        _CACHE[key] = build_program(plan)
    nc = _CACHE[key]
    res = bass_utils.run_bass_kernel_spmd(nc, plan.in_maps(),
                                          core_ids=list(range(N_CORES)))
    logits = np.concatenate([res.results[k]["out_part"] for k in range(N_CORES)], 0)
    out = 1.0 / (1.0 + np.exp(-logits.astype(np.float64)))
    return np.ascontiguousarray(out.astype(np.float32))


def kernel(**inputs) -> np.ndarray:
    return run(inputs, G=G_REAL)



# revision 2
# speedup vs baseline: 1.0684x; 1.0684x over previous
"""GCN-Tox21 GNN message-passing kernel for 8 Trainium2 NeuronCores — v2.

Restructure vs v1 (2.67ms): the per-edge MLP second layer is moved past the
segment-sum (m2 = relu(m1) @ w2 commutes with the linear scatter-sum), so the
tensor engine computes per-edge work once ([E,512] m1 only) and per-node work
20x less often. Per 128-edge subtile: 2 src matmuls (gathered h, edge-major),
1 dst matmul (per-window Q expanded via transposed one-hot), 1 edge-attr
matmul, relu, and 1 seg-sum matmul into a per-window [128,512] accumulator.
Per 128-node window: transpose the accumulator, 4 w2 matmuls + rank-2
(cnt*b2A + cntp*cb) correction, relu*(1/cnt) -> h.

DMA-instruction count (HWDGE ~625ns each, was 68% busy) cut ~7x: S/ST/eT
streamed in [*,2048] chunks, h stores batched 5 windows/DMA via a permuted
node->DRAM-row map (gather indices absorb the permutation), stage A computes
only the core's stripe and AllGathers h0 (collectives measured ~free here).
"""

import numpy as np
import ml_dtypes

import concourse.bacc as bacc
import concourse.tile as tile
from concourse import mybir, bass_utils
from concourse.masks import make_identity

BF16 = mybir.dt.bfloat16
F32 = mybir.dt.float32
F8 = mybir.dt.float8e4
RELU = mybir.ActivationFunctionType.Relu
COPY = mybir.ActivationFunctionType.Copy
DR = mybir.MatmulPerfMode.DoubleRow

N_CORES = 8
BN_EPS = 1e-5
G_REAL = 512
F_NODE, F_EDGE, H, EH = 32, 8, 256, 16
OUT_DIMS = (256, 256, 128)
F_MID = 512
EG = 512          # edges per dma_gather (hard HW limit)
CHUNK = 2048      # edges per S/ST/eT stream chunk


def _bf(a):
    return np.ascontiguousarray(a.astype(ml_dtypes.bfloat16))


def _f32(a):
    return np.ascontiguousarray(a.astype(np.float32))


def _wrap_idx(idx):
    """int16 index layout for dma_gather: index i at [i % 16, i // 16],
    replicated across the 8 partition groups."""
    assert len(idx) % 16 == 0
    w = idx.astype(np.int16).reshape(-1, 16).T
    return np.ascontiguousarray(np.tile(w, (8, 1)))


class Plan:
    """Host-side preprocessing: sharding layout + per-core input tensors."""

    def __init__(self, inputs, G):
        x = np.asarray(inputs["x"]).astype(np.float32)
        N = x.shape[0]
        self.N, self.G = N, G
        self.N_pad = ((N + N_CORES * 640 - 1) // (N_CORES * 640)) * (N_CORES * 640)
        self.NPC = self.N_pad // N_CORES
        self.W = self.NPC // 128
        self.NB = self.W // 5          # 5-window store batches per core
        assert self.W % 5 == 0
        assert G % N_CORES == 0
        self.GPC = G // N_CORES

        # node -> DRAM row permutation (batched stores write rows p*5+j)
        def row_of(n):
            n = np.asarray(n)
            k, rem = n // self.NPC, n % self.NPC
            c, r = rem // 640, rem % 640
            return k * self.NPC + c * 640 + (r % 128) * 5 + r // 128

        self.row_of = row_of

        edge_index = np.asarray(inputs["edge_index"]).astype(np.int64)
        src, dst = edge_index[0].astype(np.int32), edge_index[1].astype(np.int32)
        batch = np.asarray(inputs["batch"]).astype(np.int32)
        edge_attr = np.asarray(inputs["edge_attr"]).astype(np.float32)

        order = np.argsort(dst, kind="stable")
        s_dst, s_src = dst[order], src[order]
        s_ea = edge_attr[order]

        bounds = np.searchsorted(s_dst, np.arange(0, self.N_pad + 1, 128), "left")
        cnt_w = (bounds[1:] - bounds[:-1]).reshape(N_CORES, self.W)
        T_w = np.maximum(1, -(-cnt_w.max(axis=0) // 128))
        T_w += T_w % 2                      # even subtile count per window
        while T_w.sum() % (CHUNK // 128) != 0:
            T_w[-1] += 2
        self.T_w = [int(t) for t in T_w]
        self.T_tot = int(T_w.sum())
        self.ET = self.T_tot * 128

        cnt = np.bincount(dst, minlength=self.N_pad).astype(np.float32)
        invc_full = 1.0 / np.maximum(cnt, 1.0)
        cntp_full = np.maximum(cnt, 1.0)
        gcnt = np.bincount(batch, minlength=G).astype(np.float32)
        ginv_full = 1.0 / np.maximum(gcnt, 1.0)

        lo_k = [int(np.searchsorted(batch, k * self.GPC, "left")) for k in range(N_CORES)]
        hi_k = [int(np.searchsorted(batch, (k + 1) * self.GPC, "left")) for k in range(N_CORES)]
        self.TP = max(1, max(-(-(h - l) // 128) for l, h in zip(lo_k, hi_k)))
        self.NPOOL = self.TP * 128

        self.per_core = []
        for k in range(N_CORES):
            d = {}
            gi_src = np.zeros(self.ET, np.int32)
            ea_pad = np.zeros((self.ET, F_EDGE), np.float32)
            S = np.zeros((128, self.ET), np.float32)
            pos = 0
            for w in range(self.W):
                base = k * self.NPC + w * 128
                lo = np.searchsorted(s_dst, base, "left")
                hi = np.searchsorted(s_dst, base + 128, "left")
                n = hi - lo
                sl = slice(pos, pos + n)
                gi_src[sl] = row_of(s_src[lo:hi])
                ea_pad[sl] = s_ea[lo:hi]
                loc = (s_dst[lo:hi] - base).astype(np.int64)
                e_ids = np.arange(pos, pos + n)
                S[e_ids % 128, (e_ids // 128) * 128 + loc] = 1.0
                pos += self.T_w[w] * 128
            assert pos == self.ET

            d["gidx_src"] = _wrap_idx(gi_src)
            d["S8"] = np.ascontiguousarray(S.astype(ml_dtypes.float8_e4m3))
            ST = np.ascontiguousarray(
                S.reshape(128, self.T_tot, 128).transpose(2, 1, 0)
                .reshape(128, self.ET))
            d["S_T"] = _bf(ST)
            eaT = np.concatenate([ea_pad.T, np.ones((1, self.ET), np.float32)], 0)
            d["eaT"] = _bf(eaT)
            stripe = slice(k * self.NPC, (k + 1) * self.NPC)
            d["invc"] = _f32(invc_full[stripe].reshape(self.W, 128).T)
            d["cnt2"] = _bf(np.stack([cnt[stripe], cntp_full[stripe]], 0))
            lo, hi = lo_k[k], hi_k[k]
            pidx = np.zeros(self.NPOOL, np.int32)
            pidx[: hi - lo] = row_of(np.arange(lo, hi))
            d["pool_idx"] = _wrap_idx(pidx)
            S2 = np.zeros((128, self.TP * self.GPC), np.float32)
            pb = batch[lo:hi] - k * self.GPC
            e_ids = np.arange(hi - lo)
            S2[e_ids % 128, (e_ids // 128) * self.GPC + pb] = 1.0
            d["S2"] = _bf(S2)
            d["ginv"] = _f32(ginv_full[k * self.GPC:(k + 1) * self.GPC]
                             .reshape(self.GPC, 1))
            self.per_core.append(d)

        sh = {}
        x_pad = np.zeros((self.N_pad, F_NODE), np.float32)
        x_pad[:N] = x
        xT_full = np.concatenate([x_pad.T, np.ones((1, self.N_pad), np.float32)], 0)
        for k in range(N_CORES):
            self.per_core[k]["xT_own"] = _bf(
                xT_full[:, k * self.NPC:(k + 1) * self.NPC])
        ne_w, ne_b = _f32(inputs["ne_w"]), _f32(inputs["ne_b"])
        sh["ne_wT"] = _bf(np.concatenate([ne_w.T, ne_b[None, :]], 0))
        ee_w, ee_b = _f32(inputs["ee_w"]), _f32(inputs["ee_b"])
        sh["ee_wT"] = _bf(np.concatenate([ee_w.T, ee_b[None, :]], 0))

        in_dim = H
        self.layer_dims = []
        for i, out_dim in enumerate(OUT_DIMS):
            w1 = _f32(inputs[f"c{i}_w1"]); b1 = _f32(inputs[f"c{i}_b1"])
            w2 = _f32(inputs[f"c{i}_w2"]); b2 = _f32(inputs[f"c{i}_b2"])
            g = _f32(inputs[f"bn{i}_g"]); bb = _f32(inputs[f"bn{i}_b"])
            rm = _f32(inputs[f"bn{i}_m"]); rv = _f32(inputs[f"bn{i}_v"])
            A = g / np.sqrt(rv + BN_EPS)
            # K-order of w1: [h_dst(256), h_src(256), e(16), b1]
            sh[f"w1T_{i}"] = _bf(np.concatenate([w1.T, b1[None, :]], 0))
            sh[f"w2T_{i}"] = _bf((w2 * A[:, None]).T)
            sh[f"r2_{i}"] = _bf(np.stack([b2 * A, bb - rm * A], 0))
            self.layer_dims.append((in_dim, 2 * out_dim, out_dim))
            in_dim = out_dim

        fc_w, fc_b = _f32(inputs["fc_w"]), _f32(inputs["fc_b"])
        self.F_FC = fc_w.shape[0]
        sh["fc_wT"] = _bf(fc_w.T)
        sh["fcb_bc"] = _f32(np.tile(fc_b[None, :], (self.GPC, 1)))
        self.shared = sh

    def in_maps(self):
        return [{**self.shared, **self.per_core[k]} for k in range(N_CORES)]


def build_program(plan: Plan, n_cores=N_CORES, debug_no_collective=False,
                  repeats=1):
    nc = bacc.Bacc("TRN2", target_bir_lowering=False, debug=False,
                   num_devices=n_cores)

    ET, T_w, W, NPC = plan.ET, plan.T_w, plan.W, plan.NPC
    T_tot, NB = plan.T_tot, plan.NB
    N_pad, TP, GPC, F_FC = plan.N_pad, plan.TP, plan.GPC, plan.F_FC

    sample = plan.in_maps()[0]
    t_in = {name: nc.dram_tensor(name, list(arr.shape),
                                 mybir.dt.from_np(arr.dtype), kind="ExternalInput")
            for name, arr in sample.items()}
    out_part = nc.dram_tensor("out_part", [GPC, F_FC], F32, kind="ExternalOutput")

    # subtile -> window mapping (static)
    sub_window, sub_first, sub_last = [], [], []
    for w in range(W):
        for t in range(T_w[w]):
            sub_window.append(w)
            sub_first.append(t == 0)
            sub_last.append(t == T_w[w] - 1)

    def all_gather(h_own, h_full):
        if debug_no_collective:
            # timing-only mode (numerics wrong): tiny local copy keeps the
            # dep graph and tile allocator happy
            nc.sync.dma_start(out=h_full[0:NPC, :], in_=h_own[:])
            return
        nc.gpsimd.collective_compute(
            "AllGather", mybir.AluOpType.bypass,
            ins=[h_own.opt()], outs=[h_full.opt()],
            replica_groups=[list(range(n_cores))])

    with tile.TileContext(nc) as tc:
        with (
            tc.tile_pool(name="const", bufs=1) as cpool,
            tc.tile_pool(name="sbuf", bufs=2) as spool,
            tc.tile_pool(name="gath", bufs=8) as gpool,
            tc.tile_pool(name="m1sb", bufs=6) as m1pool,
            tc.tile_pool(name="psum", bufs=1, space="PSUM") as ppool,
            tc.tile_pool(name="dram", bufs=1, space="DRAM") as dpool,
        ):
            def _body():
                # ---------- resident constants ----------
                def load_const(name, tag=None):
                    arr = sample[name]
                    t = cpool.tile(list(arr.shape), mybir.dt.from_np(arr.dtype),
                                   tag=tag or name)
                    nc.sync.dma_start(out=t[:], in_=t_in[name][:])
                    return t

                gidx_src = load_const("gidx_src")
                pool_idx = load_const("pool_idx")
                invc_t = load_const("invc")
                cnt2_t = load_const("cnt2")
                S2_t = load_const("S2")
                ginv_t = load_const("ginv")
                ne_wT_t = load_const("ne_wT")
                ee_wT_t = load_const("ee_wT")
                fc_wT_t = load_const("fc_wT")
                fcb_t = load_const("fcb_bc")
                xo = load_const("xT_own")
                ident = cpool.tile([128, 128], BF16, tag="ident")
                make_identity(nc, ident[:])
                ones1 = cpool.tile([1, 128], BF16, tag="ones1")
                nc.vector.memset(ones1[:], 1.0)

                w1d_t, w1s_t, w1e_t, b1_t, w2T_t, r2_t = [], [], [], [], [], []
                for i, (F_in, F_mid, F_out) in enumerate(plan.layer_dims):
                    dch, sch = [], []
                    for kc in range(2):
                        td = cpool.tile([128, F_mid], BF16, tag=f"w1d_{i}_{kc}")
                        nc.sync.dma_start(
                            out=td[:], in_=t_in[f"w1T_{i}"][kc * 128:(kc + 1) * 128, :])
                        dch.append(td)
                        ts_ = cpool.tile([128, F_mid], BF16, tag=f"w1s_{i}_{kc}")
                        nc.sync.dma_start(
                            out=ts_[:],
                            in_=t_in[f"w1T_{i}"][256 + kc * 128:256 + (kc + 1) * 128, :])
                        sch.append(ts_)
                    w1d_t.append(dch)
                    w1s_t.append(sch)
                    te = cpool.tile([EH, F_mid], BF16, tag=f"w1e_{i}")
                    nc.sync.dma_start(out=te[:], in_=t_in[f"w1T_{i}"][512:528, :])
                    w1e_t.append(te)
                    tb = cpool.tile([1, F_mid], BF16, tag=f"b1_{i}")
                    nc.sync.dma_start(out=tb[:], in_=t_in[f"w1T_{i}"][528:529, :])
                    b1_t.append(tb)
                    wc = []
                    for km in range(F_mid // 128):
                        t = cpool.tile([128, F_out], BF16, tag=f"w2T_{i}_{km}")
                        nc.sync.dma_start(
                            out=t[:], in_=t_in[f"w2T_{i}"][km * 128:(km + 1) * 128, :])
                        wc.append(t)
                    w2T_t.append(wc)
                    r2_t.append(load_const(f"r2_{i}"))

                hT_t = [cpool.tile([128, W, F_in_ // 128, 128], BF16, tag=f"hT{i}",
                                   name=f"hT{i}")
                        for i, (F_in_, _, _) in enumerate(plan.layer_dims)]

                # ---------- DRAM buffers ----------
                h_full = [dpool.tile([N_pad, plan.layer_dims[0][0]], BF16,
                                     tag="h0", name="h_full0")]
                for i, (_, _, F_out) in enumerate(plan.layer_dims):
                    h_full.append(dpool.tile([N_pad, F_out], BF16, tag=f"h{i + 1}",
                                             name=f"h_full{i + 1}"))
                h_own = [dpool.tile([NPC, H], BF16, tag="hoA", name="h_ownA")]
                h_own += [dpool.tile([NPC, d[2]], BF16, tag=f"hown{i}",
                                     name=f"h_own{i}")
                          for i, d in enumerate(plan.layer_dims)]
                eT_dram = dpool.tile([EH, ET], BF16, tag="eT")

                # ---------- stage A: h0 (own stripe) + hT0 ----------
                # feature-major -> hT0 directly
                for q in range(NPC // 512):
                    for kc in range(H // 128):
                        fp = ppool.tile([128, 512], F32, tag="m1", bufs=2)
                        nc.tensor.matmul(out=fp[:],
                                         lhsT=ne_wT_t[:, kc * 128:(kc + 1) * 128],
                                         rhs=xo[:, q * 512:(q + 1) * 512],
                                         start=True, stop=True)
                        nc.scalar.activation(out=hT_t[0][:, 4 * q:4 * q + 4, kc, :],
                                             in_=fp[:], func=RELU)
                # node-major h0 -> batched store -> AllGather
                h0st = None
                for w in range(W):
                    np0 = ppool.tile([128, H], F32, tag="node", bufs=2)
                    nc.tensor.matmul(out=np0[:],
                                     lhsT=xo[:, w * 128:(w + 1) * 128],
                                     rhs=ne_wT_t[:], start=True, stop=True)
                    if w % 5 == 0:
                        h0st = spool.tile([128, 5, H], BF16, tag="hst", bufs=2)
                    nc.scalar.activation(out=h0st[:, w % 5, :], in_=np0[:],
                                         func=RELU)
                    if w % 5 == 4:
                        c = w // 5
                        nc.sync.dma_start(
                            out=h_own[0][c * 640:(c + 1) * 640, :], in_=h0st[:])
                all_gather(h_own[0], h_full[0])

                # ---------- stage A2: eT -> DRAM [EH, ET] ----------
                for c in range(ET // CHUNK):
                    ea = spool.tile([F_EDGE + 1, CHUNK], BF16, tag="ea", bufs=2)
                    nc.sync.dma_start(out=ea[:],
                                      in_=t_in["eaT"][:, c * CHUNK:(c + 1) * CHUNK])
                    ets = spool.tile([EH, CHUNK], BF16, tag="ets", bufs=2)
                    for q in range(CHUNK // 512):
                        ep = ppool.tile([EH, 512], F32, tag="qt", bufs=1)
                        nc.tensor.matmul(out=ep[:], lhsT=ee_wT_t[:],
                                         rhs=ea[:, q * 512:(q + 1) * 512],
                                         start=True, stop=True)
                        nc.scalar.activation(out=ets[:, q * 512:(q + 1) * 512],
                                             in_=ep[:], func=RELU)
                    nc.sync.dma_start(out=eT_dram[:, c * CHUNK:(c + 1) * CHUNK],
                                      in_=ets[:])

                # ---------- conv layers ----------
                for li, (F_in, F_mid, F_out) in enumerate(plan.layer_dims):
                    h_in = h_full[li]
                    gs = STc = Sc = eTc = qsb = node_ps = hst = None
                    for t in range(T_tot):
                        if t % 4 == 0:
                            e0 = t * 128
                            gs = gpool.tile([128, 2, EG], BF16, tag="gs")
                            nc.gpsimd.dma_gather(
                                gs[:], h_in[:, :],
                                gidx_src[:, e0 // 16:(e0 + EG) // 16],
                                EG, EG, F_in, transpose=True)
                        if t % 16 == 0:
                            c0 = t * 128
                            STc = spool.tile([128, CHUNK], BF16, tag="STc", bufs=2)
                            nc.sync.dma_start(out=STc[:],
                                              in_=t_in["S_T"][:, c0:c0 + CHUNK])
                            Sc = spool.tile([128, CHUNK], F8, tag="Sc", bufs=2)
                            nc.sync.dma_start(out=Sc[:],
                                              in_=t_in["S8"][:, c0:c0 + CHUNK])
                            eTc = spool.tile([EH, CHUNK], BF16, tag="eTc", bufs=2)
                            nc.sync.dma_start(out=eTc[:],
                                              in_=eT_dram[:, c0:c0 + CHUNK])
                        w = sub_window[t]
                        if sub_first[t]:
                            qp = ppool.tile([128, F_mid], F32, tag="qt", bufs=1)
                            for kc in range(2):
                                nc.tensor.matmul(
                                    out=qp[:], lhsT=hT_t[li][:, w, kc, :],
                                    rhs=w1d_t[li][kc][:],
                                    start=(kc == 0), stop=False,
                                    skip_group_check=True)
                            nc.tensor.matmul(out=qp[:], lhsT=ones1[:],
                                             rhs=b1_t[li][:], start=False,
                                             stop=True, skip_group_check=True)
                            qsb = spool.tile([128, F_mid], BF16, tag="qsb",
                                             bufs=2)
                            nc.vector.tensor_copy(out=qsb[:], in_=qp[:])
                        ge = (t % 4) * 128
                        ce = (t % 16) * 128
                        mp = ppool.tile([128, F_mid], F32, tag="m1", bufs=2)
                        for kc in range(2):
                            nc.tensor.matmul(out=mp[:],
                                             lhsT=gs[:, kc, ge:ge + 128],
                                             rhs=w1s_t[li][kc][:],
                                             start=(kc == 0), stop=False,
                                             skip_group_check=True)
                        nc.tensor.matmul(out=mp[:], lhsT=STc[:, ce:ce + 128],
                                         rhs=qsb[:], start=False, stop=False,
                                         skip_group_check=True)
                        nc.tensor.matmul(out=mp[:], lhsT=eTc[:, ce:ce + 128],
                                         rhs=w1e_t[li][:], start=False, stop=True,
                                         skip_group_check=True)
                        if t % 2 == 0:
                            m1sb = m1pool.tile([128, 2, F_mid], F8, tag="m1sb")
                            nc.scalar.activation(out=m1sb[:, 0, :], in_=mp[:],
                                                 func=RELU)
                        else:
                            nc.scalar.activation(out=m1sb[:, 1, :], in_=mp[:],
                                                 func=RELU)
                            if sub_first[t - 1]:
                                node_ps = ppool.tile([128, F_mid], F32,
                                                     tag="node", bufs=2)
                            ce2 = ((t - 1) % 16) * 128
                            nc.tensor.matmul(
                                out=node_ps[:],
                                lhsT=Sc[:, ce2:ce2 + 256]
                                .rearrange("p (j n) -> p j n", j=2),
                                rhs=m1sb[:], start=sub_first[t - 1],
                                stop=sub_last[t], perf_mode=DR,
                                skip_group_check=True)
                        if sub_last[t]:
                            ns_sb = spool.tile([128, F_mid], BF16, tag="ns",
                                               bufs=2)
                            nc.vector.tensor_copy(out=ns_sb[:], in_=node_ps[:])
                            ntp = ppool.tile([128, F_mid], BF16, tag="tp", bufs=2)
                            for kc in range(F_mid // 128):
                                nc.tensor.transpose(
                                    out=ntp[:, kc * 128:(kc + 1) * 128],
                                    in_=ns_sb[:, kc * 128:(kc + 1) * 128],
                                    identity=ident[:])
                            nsT = spool.tile([128, F_mid], BF16, tag="nsT",
                                             bufs=2)
                            nc.vector.tensor_copy(out=nsT[:], in_=ntp[:])
                            hp2 = ppool.tile([128, F_out], F32, tag="w2o",
                                             bufs=1)
                            for km in range(F_mid // 128):
                                nc.tensor.matmul(out=hp2[:],
                                                 lhsT=nsT[:, km * 128:(km + 1) * 128],
                                                 rhs=w2T_t[li][km][:],
                                                 start=(km == 0), stop=False,
                                                 skip_group_check=True)
                            nc.tensor.matmul(out=hp2[:],
                                             lhsT=cnt2_t[:, w * 128:(w + 1) * 128],
                                             rhs=r2_t[li][:], start=False,
                                             stop=True, skip_group_check=True)
                            if w % 5 == 0:
                                hst = spool.tile([128, 5, F_out], BF16,
                                                 tag="hst", bufs=2)
                            nc.scalar.activation(out=hst[:, w % 5, :], in_=hp2[:],
                                                 func=RELU,
                                                 scale=invc_t[:, w:w + 1])
                            if li < 2:
                                for kc in range(F_out // 128):
                                    tp2 = ppool.tile([128, 128], BF16, tag="tp",
                                                     bufs=2)
                                    nc.tensor.transpose(
                                        out=tp2[:],
                                        in_=hst[:, w % 5, kc * 128:(kc + 1) * 128],
                                        identity=ident[:])
                                    nc.vector.tensor_copy(
                                        out=hT_t[li + 1][:, w, kc, :], in_=tp2[:])
                            if w % 5 == 4:
                                c = w // 5
                                nc.sync.dma_start(
                                    out=h_own[li + 1][c * 640:(c + 1) * 640, :],
                                    in_=hst[:])
                    all_gather(h_own[li + 1], h_full[li + 1])

                # ---------- pooling + FC + sigmoid ----------
                F_last = plan.layer_dims[-1][2]
                hp = spool.tile([128, TP, F_last], BF16, tag="hp")
                for p0 in range(0, TP, 4):
                    pn = min(4, TP - p0)
                    nc.gpsimd.dma_gather(
                        hp[:, p0:p0 + pn, :], h_full[-1][:, :],
                        pool_idx[:, p0 * 8:(p0 + pn) * 8],
                        pn * 128, pn * 128, F_last, transpose=False)
                pool_ps = ppool.tile([GPC, F_last], F32, tag="qt", bufs=1)
                for t in range(TP):
                    nc.tensor.matmul(out=pool_ps[:],
                                     lhsT=S2_t[:, t * GPC:(t + 1) * GPC],
                                     rhs=hp[:, t, :], start=(t == 0),
                                     stop=(t == TP - 1))
                pooled_sb = spool.tile([GPC, F_last], BF16, tag="pooled")
                nc.scalar.activation(out=pooled_sb[:], in_=pool_ps[:],
                                     func=COPY, scale=ginv_t[:])
                ptr_ps = ppool.tile([F_last, GPC], BF16, tag="tp", bufs=2)
                nc.tensor.transpose(out=ptr_ps[:], in_=pooled_sb[:],
                                    identity=ident[0:GPC, 0:GPC])
                ptr_sb = spool.tile([F_last, GPC], BF16, tag="ptrsb")
                nc.vector.tensor_copy(out=ptr_sb[:], in_=ptr_ps[:])
                fc_ps = ppool.tile([GPC, F_FC], F32, tag="w2o", bufs=1)
                nc.tensor.matmul(out=fc_ps[:], lhsT=ptr_sb[:], rhs=fc_wT_t[:],
                                 start=True, stop=True)
                logit = spool.tile([GPC, F_FC], F32, tag="logit")
                nc.vector.tensor_tensor(out=logit[:], in0=fc_ps[:], in1=fcb_t[:],
                                        op=mybir.AluOpType.add)
                # host applies the exact fp32 sigmoid
                nc.sync.dma_start(out=out_part[:], in_=logit[:])

            for _r in range(repeats):
                _body()

    nc.compile()
    return nc


_CACHE = {}


def run(inputs, G=G_REAL):
    plan = Plan(inputs, G)
    key = (plan.N, plan.G, plan.TP, tuple(plan.T_w))
    if key not in _CACHE:
        _CACHE[key] = build_program(plan)
    nc = _CACHE[key]
    res = bass_utils.run_bass_kernel_spmd(nc, plan.in_maps(),
                                          core_ids=list(range(N_CORES)))
    logits = np.concatenate([res.results[k]["out_part"] for k in range(N_CORES)], 0)
    out = 1.0 / (1.0 + np.exp(-logits.astype(np.float64)))
    return np.ascontiguousarray(out.astype(np.float32))


def kernel(**inputs) -> np.ndarray:
    return run(inputs, G=G_REAL)
